# revision 8
# baseline (speedup 1.0000x reference)
"""KAN-attention kernel for 8 Trainium2 NeuronCores.

Math: y[b,o] = sum_i [ sb[o,i]*silu(x[b,i]) + sp[o,i]*sum_c B[b,i,c]*coef[o,i,c] ]
for the q and k branches summed, + bias, softmax over groups of out_dim=8.

Strategy: shard act_out (o = 2048) into 8 contiguous slices of 256 (tensor
parallel). Softmax groups (8) never straddle a slice -> zero collectives.
Per core this is one big matmul with contraction K = (c, i): stationary =
B-spline features [K,8] (tiny, host-computed), moving = sp (.) coef [K, 256].
coef is host-transposed to (c, ip, iblk, o) so every DMA is 16KB/partition
contiguous; the sp multiply runs on DVE in-place; base term uses plain fp32
matmul (exactness), spline blocks use float32r (full PE rate at N>=256).
"""

import numpy as np

NUM_F, POLY = 5, 3
C = NUM_F + POLY          # 8 spline coefficients per edge
BATCH, HEADS, PATCHES, DIM, OUT_DIM = 8, 4, 64, 8, 8
ACT_IN = HEADS * PATCHES * DIM      # 2048
ACT_OUT = HEADS * PATCHES * OUT_DIM  # 2048
N_CORES = 8
O_PER = ACT_OUT // N_CORES          # 256
IBLK, IP = 16, 128                  # i = iblk*128 + ip


def _ext_grid():
    g = np.linspace(-1.0, 1.0, NUM_F + 1)
    h = 2.0 / NUM_F
    left = g[0] - h * np.arange(POLY, 0, -1)
    right = g[-1] + h * np.arange(1, POLY + 1)
    return np.concatenate([left, g, right]).astype(np.float32)  # (12,)


def _bspline_basis(x, grid, k):
    # Cox-de Boor; x: (..., ), grid: (G,) -> (..., G-1-k)
    x1 = x[..., None]
    B = ((x1 >= grid[:-1]) & (x1 < grid[1:])).astype(x.dtype)
    for d in range(1, k + 1):
        left = (x1 - grid[:-(d + 1)]) / (grid[d:-1] - grid[:-(d + 1)])
        right = (grid[d + 1:] - x1) / (grid[d + 1:] - grid[1:-d])
        B = left * B[..., :-1] + right * B[..., 1:]
    return B


_NC_CACHE = []


def _build_program():
    if _NC_CACHE:
        return _NC_CACHE[0]
    from contextlib import ExitStack

    import concourse.bacc as bacc
    import concourse.tile as tile
    from concourse import mybir

    dt = mybir.dt
    f32 = dt.float32
    f32r = dt.float32r

    nc = bacc.Bacc(target_bir_lowering=False, trn_type="TRN2", debug=False)

    cq = nc.dram_tensor("cq", [C, IP, IBLK, O_PER], f32, kind="ExternalInput").ap()
    ck = nc.dram_tensor("ck", [C, IP, IBLK, O_PER], f32, kind="ExternalInput").ap()
    mq = nc.dram_tensor("mq", [IP, IBLK, O_PER], f32, kind="ExternalInput").ap()
    mk = nc.dram_tensor("mk", [IP, IBLK, O_PER], f32, kind="ExternalInput").ap()
    ssp = nc.dram_tensor("ssp", [IP, IBLK, O_PER], f32, kind="ExternalInput").ap()
    sbs = nc.dram_tensor("sbs", [IP, IBLK, O_PER], f32, kind="ExternalInput").ap()
    fq = nc.dram_tensor("fq", [IP, C + 1, IBLK, BATCH], f32, kind="ExternalInput").ap()
    fk = nc.dram_tensor("fk", [IP, C + 1, IBLK, BATCH], f32, kind="ExternalInput").ap()
    bias = nc.dram_tensor("bias", [BATCH, O_PER], f32, kind="ExternalInput").ap()
    yout = nc.dram_tensor("yout", [IP, 2, OUT_DIM], f32, kind="ExternalOutput").ap()

    with tile.TileContext(nc) as tc, ExitStack() as ctx:
        feats = ctx.enter_context(tc.tile_pool(name="feats", bufs=1))
        scales = ctx.enter_context(tc.tile_pool(name="scales", bufs=1))
        masks = ctx.enter_context(tc.tile_pool(name="masks", bufs=1))
        sps = ctx.enter_context(tc.tile_pool(name="sps", bufs=2))
        coefs = ctx.enter_context(tc.tile_pool(name="coefs", bufs=3))
        ws = ctx.enter_context(tc.tile_pool(name="ws", bufs=3))
        outp = ctx.enter_context(tc.tile_pool(name="outp", bufs=1))
        psum = ctx.enter_context(tc.tile_pool(name="psum", bufs=1, space="PSUM"))

        fq_sb = feats.tile([IP, C + 1, IBLK, BATCH], f32)
        fk_sb = feats.tile([IP, C + 1, IBLK, BATCH], f32)
        nc.sync.dma_start(out=fq_sb[:], in_=fq[:])
        nc.sync.dma_start(out=fk_sb[:], in_=fk[:])
        # float32r copies of the spline features (fp32r matmul: ~2 cyc/row
        # warm, near-fp32r accuracy; bf16 was 11x worse on rel err)
        fqr_sb = feats.tile([IP, C, IBLK, BATCH], f32r)
        fkr_sb = feats.tile([IP, C, IBLK, BATCH], f32r)
        nc.vector.tensor_copy(out=fqr_sb[:], in_=fq_sb[:, :C])
        nc.vector.tensor_copy(out=fkr_sb[:], in_=fk_sb[:, :C])

        ssp_sb = scales.tile([IP, IBLK, O_PER], f32)
        sbs_sb = scales.tile([IP, IBLK, O_PER], f32)
        nc.sync.dma_start(out=ssp_sb[:], in_=ssp[:])
        nc.sync.dma_start(out=sbs_sb[:], in_=sbs[:])

        bias_sb = outp.tile([BATCH, O_PER], f32)
        nc.sync.dma_start(out=bias_sb[:], in_=bias[:])

        Y = psum.tile([BATCH, O_PER], f32)

        n_mm = 2 * (IBLK + C * IBLK)  # 288
        mm = [0]

        def flags():
            i = mm[0]
            mm[0] += 1
            return dict(start=(i == 0), stop=(i == n_mm - 1))

        for m_dram, c_dram, f_sb, fr_sb in (
            (mq, cq, fq_sb, fqr_sb), (mk, ck, fk_sb, fkr_sb)
        ):
            m_sb = masks.tile([IP, IBLK, O_PER], f32, tag="mask")
            nc.sync.dma_start(out=m_sb[:], in_=m_dram[:])

            # base term: weights sb = mask * scale_base, plain fp32 matmul
            sb_w = ws.tile([IP, IBLK, O_PER], f32, tag="w")
            nc.vector.tensor_mul(sb_w[:], m_sb[:], sbs_sb[:])
            for ib in range(IBLK):
                nc.tensor.matmul(Y[:], f_sb[:, C, ib, :], sb_w[:, ib, :], **flags())

            # spline weights sp = mask * scale_sp, reused by all 8 c-blocks
            sp_sb = sps.tile([IP, IBLK, O_PER], f32, tag="sp")
            nc.vector.tensor_mul(sp_sb[:], m_sb[:], ssp_sb[:])

            for c in range(C):
                cb = coefs.tile([IP, IBLK, O_PER], f32, tag="coef")
                nc.sync.dma_start(out=cb[:], in_=c_dram[c])
                w = ws.tile([IP, IBLK, O_PER], f32r, tag="w")
                # split the big elementwise stream across DVE and GpSimd
                eng = nc.gpsimd if c in (2, 5, 7) else nc.vector
                eng.tensor_tensor(w[:], cb[:], sp_sb[:], mybir.AluOpType.mult)
                for ib in range(IBLK):
                    nc.tensor.matmul(
                        Y[:], fr_sb[:, c, ib, :], w[:, ib, :], **flags(),
                    )

        # y = Y + bias, then regroup to [(b%4)*32+g, b//4, d] for the softmax
        ybuf = outp.tile([BATCH, O_PER], f32)
        nc.vector.tensor_add(ybuf[:], Y[:], bias_sb[:])

        yt = outp.tile([IP, 2, OUT_DIM], f32)
        for h in range(2):
            src = ybuf[4 * h:4 * h + 4, :].rearrange("b (g d) -> b g d", d=OUT_DIM)
            nc.sync.dma_start(out=yt[:, h, :], in_=src)

        # softmax over d within each partition row: exp(y - max) / sum
        mx = outp.tile([IP, 2], f32)
        sm = outp.tile([IP, 2], f32)
        for h in range(2):
            nc.vector.tensor_reduce(
                mx[:, h:h + 1], yt[:, h, :],
                axis=mybir.AxisListType.X, op=mybir.AluOpType.max, negate=True,
            )
            nc.scalar.activation(
                yt[:, h, :], yt[:, h, :],
                mybir.ActivationFunctionType.Exp, bias=mx[:, h:h + 1],
            )
            nc.vector.tensor_reduce(
                sm[:, h:h + 1], yt[:, h, :],
                axis=mybir.AxisListType.X, op=mybir.AluOpType.add,
            )
        nc.vector.reciprocal(sm[:], sm[:])
        for h in range(2):
            nc.vector.tensor_scalar_mul(yt[:, h, :], yt[:, h, :], sm[:, h:h + 1])

        nc.sync.dma_start(out=yout[:], in_=yt[:])

    nc.compile()
    _NC_CACHE.append(nc)
    return nc


def _host_prep(q, k, coef_q, coef_k, scale_base, scale_sp, mask_q, mask_k, bias_w):
    grid = _ext_grid()
    xq = np.ascontiguousarray(q, np.float32).reshape(BATCH, ACT_IN)
    xk = np.ascontiguousarray(k, np.float32).reshape(BATCH, ACT_IN)

    def feat(x):
        B = _bspline_basis(x, grid, POLY)            # (8, 2048, 8)
        silu = (x / (1.0 + np.exp(-x))).astype(np.float32)
        fb = B.reshape(BATCH, IBLK, IP, C).transpose(2, 3, 1, 0)   # (128,8,16,8)
        fs = silu.reshape(BATCH, IBLK, IP).transpose(2, 1, 0)      # (128,16,8)
        return np.ascontiguousarray(
            np.concatenate([fb, fs[:, None]], axis=1), np.float32)  # (128,9,16,8)

    fq_h, fk_h = feat(xq), feat(xk)

    def wslices(t):  # (N,...) over edges n = o*2048+i -> per-core (.., 128, 16, 256)
        t = np.asarray(t, np.float32)
        if t.ndim == 1:  # scale/mask: (N,) -> (m, 128, 16, 256)
            v = t.reshape(N_CORES, O_PER, IBLK, IP).transpose(0, 3, 2, 1)
        else:            # coef: (N, C) -> (m, C, 128, 16, 256)
            v = t.reshape(N_CORES, O_PER, IBLK, IP, C).transpose(0, 4, 3, 2, 1)
        return np.ascontiguousarray(v)

    cq_h, ck_h = wslices(coef_q), wslices(coef_k)
    mq_h, mk_h = wslices(mask_q), wslices(mask_k)
    ssp_h, sbs_h = wslices(scale_sp), wslices(scale_base)
    bias_h = np.asarray(bias_w, np.float32).reshape(N_CORES, 1, O_PER)

    in_maps = []
    for m in range(N_CORES):
        in_maps.append({
            "cq": cq_h[m], "ck": ck_h[m],
            "mq": mq_h[m], "mk": mk_h[m],
            "ssp": ssp_h[m], "sbs": sbs_h[m],
            "fq": fq_h, "fk": fk_h,
            "bias": np.ascontiguousarray(np.broadcast_to(bias_h[m], (BATCH, O_PER))),
        })
    return in_maps


def _assemble(results):
    # yout [128, 2, 8]: partition p = b_lo*32 + g, free = (b_hi, d); b = b_hi*4+b_lo
    out = np.empty((BATCH, HEADS, PATCHES, OUT_DIM), np.float32)
    flat = out.reshape(BATCH, ACT_OUT)
    for m, r in enumerate(results):
        y = r["yout"].reshape(4, 32, 2, OUT_DIM)          # (b_lo, g, b_hi, d)
        y = y.transpose(2, 0, 1, 3).reshape(BATCH, O_PER)  # (b, g*8+d)
        flat[:, m * O_PER:(m + 1) * O_PER] = y
    return out


def kernel(q, k, coef_q, coef_k, scale_base, scale_sp, mask_q, mask_k, bias_w,
           _trace=False):
    from concourse.bass_utils import run_bass_kernel_spmd

    nc = _build_program()
    in_maps = _host_prep(q, k, coef_q, coef_k, scale_base, scale_sp,
                         mask_q, mask_k, bias_w)
    res = run_bass_kernel_spmd(nc, in_maps, core_ids=list(range(N_CORES)),
                               trace=_trace)
    out = _assemble(res.results)
    if _trace:
        return out, res
    return out
